# revision 3
# baseline (speedup 1.0000x reference)
"""DeepseekV3.2 indexer — trn2 Bass kernel (8 NeuronCores, sequence-parallel).

Self-contained: builds the Bass program, shards the full inputs over 8 cores
(queries sharded; K-path inputs replicated), runs via run_bass_kernel_spmd,
and reassembles the full [1, 2048, 2048] int32 top-k index output.

Device pipeline per core (256 queries):
  S1  q_resid^T via PE transposes
  S2  k-proj for all T + head-weight-proj for own rows (streamed hidden tiles,
      PE transpose + fp32 matmul accumulation, one psum group per bank)
  S3  LayerNorm (2-pass, newton-refined sqrt) + rope on k; transpose -> kT
  S3b head-weight pair-scale table
  per s-tile (128 queries):
    S4  q-proj (fp32) + rope + per-head transpose -> qT
    S5  64 query pairs: scores matmul -> ACT relu(scale=hw) -> ones-matmul
        head-sum accumulated into a [128, T] psum tile (placement trick)
    S6  exact sort: per-128-segment max8 extraction + bitonic merge network
        (values + uint32 index payload), DMA indices out
"""
from contextlib import ExitStack
from dataclasses import dataclass

import numpy as np

import concourse.bass as bass
import concourse.mybir as mybir
import concourse.tile as tile
from concourse import bacc
from concourse.bass import ts
from concourse.bass_utils import run_bass_kernel_spmd

Alu = mybir.AluOpType
Act = mybir.ActivationFunctionType
F32 = mybir.dt.float32
U32 = mybir.dt.uint32
LN_EPS = 1e-5


@dataclass
class Cfg:
    T: int = 2048
    Ssh: int = 256
    HID: int = 7168
    QLR: int = 1536
    H: int = 64
    D: int = 128
    DR: int = 64
    SEG: int = 128
    n_cores: int = 8
    debug: bool = False


def build(cfg: Cfg):
    T, Ssh, HID, QLR, H, D, DR = cfg.T, cfg.Ssh, cfg.HID, cfg.QLR, cfg.H, cfg.D, cfg.DR
    KT = HID // 128
    QKT = QLR // 128
    NT = T // 128
    NS = Ssh // 128
    NPAIR = 64
    HD = H * D
    NPB = T // 512

    nc = bacc.Bacc("TRN2", target_bir_lowering=False, debug=False,
                   num_devices=cfg.n_cores)

    hidden = nc.dram_tensor("hidden", [T, HID], F32, kind="ExternalInput").ap()
    hidden_o = nc.dram_tensor("hidden_o", [Ssh, HID], F32, kind="ExternalInput").ap()
    q_resid_o = nc.dram_tensor("q_resid_o", [Ssh, QLR], F32, kind="ExternalInput").ap()
    cos_o = nc.dram_tensor("cos_o", [Ssh, DR // 2], F32, kind="ExternalInput").ap()
    sin_o = nc.dram_tensor("sin_o", [Ssh, DR // 2], F32, kind="ExternalInput").ap()
    cos_f = nc.dram_tensor("cos_f", [T, DR // 2], F32, kind="ExternalInput").ap()
    sin_f = nc.dram_tensor("sin_f", [T, DR // 2], F32, kind="ExternalInput").ap()
    Wq = nc.dram_tensor("Wq", [QLR, HD], F32, kind="ExternalInput").ap()
    Wk = nc.dram_tensor("Wk", [HID, D], F32, kind="ExternalInput").ap()
    Ww = nc.dram_tensor("Ww", [HID, H], F32, kind="ExternalInput").ap()
    ln_g = nc.dram_tensor("ln_g", [128, D], F32, kind="ExternalInput").ap()
    ln_b = nc.dram_tensor("ln_b", [128, D], F32, kind="ExternalInput").ap()
    ones2 = nc.dram_tensor("ones2", [128, 256], F32, kind="ExternalInput").ap()
    ident = nc.dram_tensor("ident", [128, 128], F32, kind="ExternalInput").ap()
    topk_o = nc.dram_tensor("topk_o", [Ssh, T], U32, kind="ExternalOutput").ap()
    if cfg.debug:
        dbg_k = nc.dram_tensor("dbg_k", [128, NT * D], F32, kind="ExternalOutput").ap()
        dbg_hwp = nc.dram_tensor("dbg_hwp", [128, NS * 64], F32, kind="ExternalOutput").ap()
        dbg_is = nc.dram_tensor("dbg_is", [NS * 128, T], F32, kind="ExternalOutput").ap()

    with tile.TileContext(nc) as tc, ExitStack() as ctx:
        const = ctx.enter_context(tc.tile_pool(name="const", bufs=1))
        big = ctx.enter_context(tc.tile_pool(name="big", bufs=1))

        ident_sb = const.tile([128, 128], F32, tag="ident")
        nc.sync.dma_start(ident_sb[:], ident)
        ones2_sb = const.tile([128, 256], F32, tag="ones2")
        nc.sync.dma_start(ones2_sb[:], ones2)
        lng_sb = const.tile([128, D], F32, tag="lng")
        nc.sync.dma_start(lng_sb[:], ln_g)
        lnb_sb = const.tile([128, D], F32, tag="lnb")
        nc.sync.dma_start(lnb_sb[:], ln_b)
        wk_sb = const.tile([128, KT * D], F32, tag="wk")
        nc.sync.dma_start(wk_sb[:].rearrange("p (kt d) -> p kt d", d=D),
                          Wk.rearrange("(kt p) d -> p kt d", p=128))
        ww_sb = const.tile([128, KT * H], F32, tag="ww")
        nc.sync.dma_start(ww_sb[:].rearrange("p (kt h) -> p kt h", h=H),
                          Ww.rearrange("(kt p) h -> p kt h", p=128))
        coso_sb = const.tile([128, NS * (DR // 2)], F32, tag="coso")
        nc.sync.dma_start(coso_sb[:].rearrange("p (st d) -> p st d", st=NS),
                          cos_o.rearrange("(st p) d -> p st d", p=128))
        sino_sb = const.tile([128, NS * (DR // 2)], F32, tag="sino")
        nc.sync.dma_start(sino_sb[:].rearrange("p (st d) -> p st d", st=NS),
                          sin_o.rearrange("(st p) d -> p st d", p=128))
        cosf_sb = const.tile([128, NT * (DR // 2)], F32, tag="cosf")
        nc.sync.dma_start(cosf_sb[:].rearrange("p (tt d) -> p tt d", tt=NT),
                          cos_f.rearrange("(tt p) d -> p tt d", p=128))
        sinf_sb = const.tile([128, NT * (DR // 2)], F32, tag="sinf")
        nc.sync.dma_start(sinf_sb[:].rearrange("p (tt d) -> p tt d", tt=NT),
                          sin_f.rearrange("(tt p) d -> p tt d", p=128))
        qr_sb = const.tile([128, NS * QLR], F32, tag="qr")
        nc.sync.dma_start(qr_sb[:].rearrange("p (st q) -> p st q", st=NS),
                          q_resid_o.rearrange("(st p) q -> p st q", p=128))

        # ---- S1: q_resid^T ----
        qrT = const.tile([128, QKT * NS * 128], F32, tag="qrT")
        with tc.tile_pool(name="ps_t1", bufs=2, space="PSUM") as ps_t1:
            for kt in range(QKT):
                for st in range(NS):
                    pt = ps_t1.tile([128, 128], F32, tag="t1")
                    nc.tensor.transpose(pt[:], qr_sb[:, st * QLR + kt * 128: st * QLR + (kt + 1) * 128], ident_sb[:])
                    nc.scalar.copy(qrT[:, kt * (NS * 128) + st * 128: kt * (NS * 128) + (st + 1) * 128], pt[:])

        # ---- S2: k-proj + hw-proj ----
        k_nat = big.tile([128, NT * D], F32, tag="k_nat")
        hw_nat = big.tile([128, NS * H], F32, tag="hw_nat")
        with (
            tc.tile_pool(name="hidl", bufs=4) as hidl,
            tc.tile_pool(name="hidTs", bufs=4) as hidTs,
            tc.tile_pool(name="ps_tr", bufs=2, space="PSUM") as ps_tr,
            tc.tile_pool(name="ps_k", bufs=2, space="PSUM") as ps_k,
        ):
            for tt in range(NT):
                kpsum = ps_k.tile([128, D], F32, tag="kpsum")
                for kt in range(KT):
                    hidtile = hidl.tile([128, 128], F32, tag="hidtile")
                    nc.sync.dma_start(hidtile[:], hidden[ts(tt, 128), ts(kt, 128)])
                    ptr = ps_tr.tile([128, 128], F32, tag="tr")
                    nc.tensor.transpose(ptr[:], hidtile[:], ident_sb[:])
                    hT = hidTs.tile([128, 128], F32, tag="hT")
                    nc.scalar.copy(hT[:], ptr[:])
                    nc.tensor.matmul(kpsum[:], hT[:], wk_sb[:, ts(kt, D)],
                                     start=(kt == 0), stop=(kt == KT - 1))
                nc.scalar.copy(k_nat[:, ts(tt, D)], kpsum[:])
            for st in range(NS):
                hwpsum = ps_k.tile([128, H], F32, tag="hwpsum")
                for kt in range(KT):
                    hidtile = hidl.tile([128, 128], F32, tag="hidtile")
                    nc.sync.dma_start(hidtile[:], hidden_o[ts(st, 128), ts(kt, 128)])
                    ptr = ps_tr.tile([128, 128], F32, tag="tr")
                    nc.tensor.transpose(ptr[:], hidtile[:], ident_sb[:])
                    hT = hidTs.tile([128, 128], F32, tag="hT")
                    nc.scalar.copy(hT[:], ptr[:])
                    nc.tensor.matmul(hwpsum[:], hT[:], ww_sb[:, ts(kt, H)],
                                     start=(kt == 0), stop=(kt == KT - 1))
                nc.scalar.copy(hw_nat[:, ts(st, H)], hwpsum[:])

        # ---- S3: LN + rope on k; transpose -> kT ----
        kTt = big.tile([128, T], F32, tag="kTt")
        with (
            tc.tile_pool(name="lnp", bufs=2) as lnp,
            tc.tile_pool(name="ps_kt", bufs=2, space="PSUM") as ps_kt,
        ):
            for tt in range(NT):
                kt_sl = k_nat[:, ts(tt, D)]
                mean = lnp.tile([128, 1], F32, tag="mean")
                nc.vector.tensor_reduce(mean[:], kt_sl, axis=mybir.AxisListType.X, op=Alu.add)
                nc.vector.tensor_scalar_mul(mean[:], mean[:], 1.0 / D)
                xc = lnp.tile([128, D], F32, tag="xc")
                nc.vector.tensor_scalar(xc[:], kt_sl, mean[:], None, op0=Alu.subtract)
                sq = lnp.tile([128, D], F32, tag="sq")
                nc.vector.tensor_tensor(sq[:], xc[:], xc[:], op=Alu.mult)
                var = lnp.tile([128, 1], F32, tag="var")
                nc.vector.tensor_reduce(var[:], sq[:], axis=mybir.AxisListType.X, op=Alu.add)
                nc.vector.tensor_scalar_mul(var[:], var[:], 1.0 / D)
                vpe = lnp.tile([128, 1], F32, tag="vpe")
                nc.vector.tensor_scalar_add(vpe[:], var[:], LN_EPS)
                s0 = lnp.tile([128, 1], F32, tag="s0")
                nc.scalar.activation(s0[:], vpe[:], Act.Sqrt)
                r0 = lnp.tile([128, 1], F32, tag="r0")
                nc.vector.reciprocal(r0[:], s0[:])
                tnum = lnp.tile([128, 1], F32, tag="tnum")
                nc.vector.tensor_tensor(tnum[:], vpe[:], r0[:], op=Alu.mult)
                s1 = lnp.tile([128, 1], F32, tag="s1")
                nc.vector.tensor_tensor(s1[:], s0[:], tnum[:], op=Alu.add)
                nc.vector.tensor_scalar_mul(s1[:], s1[:], 0.5)
                rinv = lnp.tile([128, 1], F32, tag="rinv")
                nc.vector.reciprocal(rinv[:], s1[:])
                nc.vector.tensor_scalar(xc[:], xc[:], rinv[:], None, op0=Alu.mult)
                nc.vector.tensor_tensor(xc[:], xc[:], lng_sb[:], op=Alu.mult)
                nc.vector.tensor_tensor(kt_sl, xc[:], lnb_sb[:], op=Alu.add)
                half = DR // 2
                c = cosf_sb[:, ts(tt, half)]
                s = sinf_sb[:, ts(tt, half)]
                xr = lnp.tile([128, half], F32, tag="xr")
                xi = lnp.tile([128, half], F32, tag="xi")
                nc.vector.tensor_copy(xr[:], kt_sl[:, 0:half])
                nc.vector.tensor_copy(xi[:], kt_sl[:, half:DR])
                t1 = lnp.tile([128, half], F32, tag="t1")
                t2 = lnp.tile([128, half], F32, tag="t2")
                nc.vector.tensor_tensor(t1[:], xr[:], c, op=Alu.mult)
                nc.vector.tensor_tensor(t2[:], xi[:], s, op=Alu.mult)
                nc.vector.tensor_tensor(kt_sl[:, 0:half], t1[:], t2[:], op=Alu.subtract)
                nc.vector.tensor_tensor(t1[:], xr[:], s, op=Alu.mult)
                nc.vector.tensor_tensor(t2[:], xi[:], c, op=Alu.mult)
                nc.vector.tensor_tensor(kt_sl[:, half:DR], t1[:], t2[:], op=Alu.add)
                pkt = ps_kt.tile([128, 128], F32, tag="pkt")
                nc.tensor.transpose(pkt[:], kt_sl, ident_sb[:])
                nc.scalar.copy(kTt[:, ts(tt, 128)], pkt[:])

        # ---- S3b: hw_pairs ----
        hw_pairs = big.tile([128, NS * 64], F32, tag="hw_pairs")
        with tc.tile_pool(name="ps_hw", bufs=2, space="PSUM") as ps_hw:
            hwT = big.tile([64, NS * 128], F32, tag="hwT")
            for st in range(NS):
                ph = ps_hw.tile([128, 128], F32, tag="ph")
                nc.tensor.transpose(ph[:64, :], hw_nat[:, ts(st, 64)], ident_sb[:])
                nc.scalar.copy(hwT[:, ts(st, 128)], ph[:64, :])
            for st in range(NS):
                src = hwT[:].rearrange("h (st2 p two) -> h st2 p two", st2=NS, two=2)
                nc.sync.dma_start(hw_pairs[0:64, ts(st, 64)], src[:, st, :, 0])
                nc.sync.dma_start(hw_pairs[64:128, ts(st, 64)], src[:, st, :, 1])

        if cfg.debug:
            nc.sync.dma_start(dbg_k, k_nat[:])
            nc.sync.dma_start(dbg_hwp, hw_pairs[:])

        # ---- per s-tile: S4 q-proj, S5 scores, S6 sort ----
        NSEG = T // cfg.SEG
        W = cfg.SEG
        for st in range(NS):
            with tc.tile_pool(name=f"qTp{st}", bufs=1) as qTp:
                qT = qTp.tile([128, H * 128], F32, tag="qT", name=f"qT{st}")
                # S4
                with (
                    tc.tile_pool(name=f"wql{st}", bufs=3) as wql,
                    tc.tile_pool(name=f"qnatp{st}", bufs=1) as qnatp,
                    tc.tile_pool(name=f"ps_q{st}", bufs=2, space="PSUM") as ps_q,
                    tc.tile_pool(name=f"ps_qt{st}", bufs=2, space="PSUM") as ps_qt,
                ):
                    q_nat = qnatp.tile([128, HD], F32, tag="q_nat", name=f"q_nat{st}")
                    for nb in range(HD // 512):
                        pq = ps_q.tile([128, 512], F32, tag="pq")
                        for kt in range(QKT):
                            wq_t = wql.tile([128, 512], F32, tag="wq")
                            nc.sync.dma_start(wq_t[:], Wq[ts(kt, 128), ts(nb, 512)])
                            nc.tensor.matmul(pq[:], qrT[:, kt * (NS * 128) + st * 128: kt * (NS * 128) + (st + 1) * 128],
                                             wq_t[:], start=(kt == 0), stop=(kt == QKT - 1))
                        nc.scalar.copy(q_nat[:, ts(nb, 512)], pq[:])
                    half = DR // 2
                    qv = q_nat[:].rearrange("p (h d) -> p h d", h=H)
                    xr = qnatp.tile([128, H * half], F32, tag="qxr", name=f"qxr{st}")
                    xi = qnatp.tile([128, H * half], F32, tag="qxi", name=f"qxi{st}")
                    xrv = xr[:].rearrange("p (h d) -> p h d", h=H)
                    xiv = xi[:].rearrange("p (h d) -> p h d", h=H)
                    nc.vector.tensor_copy(xrv, qv[:, :, 0:half])
                    nc.vector.tensor_copy(xiv, qv[:, :, half:DR])
                    c = coso_sb[:, ts(st, half)].unsqueeze(1).to_broadcast([128, H, half])
                    s = sino_sb[:, ts(st, half)].unsqueeze(1).to_broadcast([128, H, half])
                    tq = qnatp.tile([128, H * half], F32, tag="qtt", name=f"qtt{st}")
                    tqv = tq[:].rearrange("p (h d) -> p h d", h=H)
                    nc.vector.tensor_tensor(tqv, xiv, s, op=Alu.mult)
                    nc.vector.tensor_tensor(qv[:, :, 0:half], xrv, c, op=Alu.mult)
                    nc.vector.tensor_tensor(qv[:, :, 0:half], qv[:, :, 0:half], tqv, op=Alu.subtract)
                    nc.vector.tensor_tensor(tqv, xiv, c, op=Alu.mult)
                    nc.vector.tensor_tensor(qv[:, :, half:DR], xrv, s, op=Alu.mult)
                    nc.vector.tensor_tensor(qv[:, :, half:DR], qv[:, :, half:DR], tqv, op=Alu.add)
                    # qT layout: [d, (s q, h)] -> column s*H + h (pair block contiguous)
                    for hg in range(H // 4):
                        pqt = ps_qt.tile([128, 512], F32, tag="pqt")
                        for j in range(4):
                            h = hg * 4 + j
                            nc.tensor.transpose(pqt[:, ts(j, 128)], q_nat[:, ts(h, D)], ident_sb[:])
                        for j in range(4):
                            h = hg * 4 + j
                            dst = qT[:].rearrange("d (s h) -> d s h", h=H)[:, :, h]
                            nc.scalar.copy(dst, pqt[:, ts(j, 128)])
                # S5
                with (
                    tc.tile_pool(name=f"ps_sc{st}", bufs=4, space="PSUM") as ps_sc,
                    tc.tile_pool(name=f"ps_is{st}", bufs=1, space="PSUM") as ps_is,
                    tc.tile_pool(name=f"relup{st}", bufs=2) as relup,
                ):
                    isp = [ps_is.tile([128, 512], F32, tag=f"isp{b}", name=f"isp{st}_{b}")
                           for b in range(NPB)]
                    for p in range(NPAIR):
                        gp = st * 64 + p
                        lhs = qT[:, 2 * p * H: (2 * p + 2) * H]
                        relu_t = relup.tile([128, T], F32, tag="relu")
                        for b in range(NPB):
                            psc = ps_sc.tile([128, 512], F32, tag="psc")
                            nc.tensor.matmul(psc[:], lhs, kTt[:, ts(b, 512)], start=True, stop=True)
                            nc.scalar.activation(relu_t[:, ts(b, 512)], psc[:], Act.Relu,
                                                 scale=hw_pairs[:, gp:gp + 1])
                            nc.tensor.matmul(isp[b][:], ones2_sb[:, 128 - 2 * p: 256 - 2 * p],
                                             relu_t[:, ts(b, 512)],
                                             start=(p == 0), stop=(p == NPAIR - 1))
                    # S6
                    with tc.tile_pool(name=f"sortp{st}", bufs=1) as sp:
                        work = sp.tile([128, T], F32, tag="work", name=f"work{st}")
                        for b in range(NPB):
                            nc.vector.tensor_copy(work[:, ts(b, 512)], isp[b][:])
                        if cfg.debug:
                            nc.sync.dma_start(dbg_is.rearrange("(st2 p) t -> st2 p t", p=128)[st], work[:])
                        sv = sp.tile([128, T], F32, tag="sv", name=f"sv{st}")
                        si = sp.tile([128, T], U32, tag="si", name=f"si{st}")
                        for seg in range(NSEG):
                            segw = work[:, ts(seg, W)]
                            for i in range(W // 8):
                                o = seg * W + i * 8
                                nc.vector.max(sv[:, o:o + 8], segw)
                                nc.vector.max_index(si[:, o:o + 8], sv[:, o:o + 8], segw)
                                nc.vector.match_replace(segw, in_to_replace=sv[:, o:o + 8],
                                                        in_values=segw, imm_value=-1e30)
                            if seg:
                                nc.vector.tensor_scalar_add(si[:, ts(seg, W)], si[:, ts(seg, W)], float(seg * W))
                        sv2 = sp.tile([128, T], F32, tag="sv2", name=f"sv2{st}")
                        si2 = sp.tile([128, T], U32, tag="si2", name=f"si2{st}")
                        cmp = sp.tile([128, T], mybir.dt.uint8, tag="cmp", name=f"cmp{st}")
                        cur_v, cur_i, alt_v, alt_i = sv, si, sv2, si2
                        L = W
                        while L < T:
                            stages = [(L, True)]
                            d = L // 2
                            while d >= 1:
                                stages.append((d, False))
                                d //= 2
                            for d, reflect in stages:
                                av = cur_v[:].rearrange("p (n two l) -> p n two l", two=2, l=d)
                                ai = cur_i[:].rearrange("p (n two l) -> p n two l", two=2, l=d)
                                ov = alt_v[:].rearrange("p (n two l) -> p n two l", two=2, l=d)
                                oi = alt_i[:].rearrange("p (n two l) -> p n two l", two=2, l=d)
                                cm = cmp[:].rearrange("p (n two l) -> p n two l", two=2, l=d)[:, :, 0, :]
                                if reflect:
                                    a_v, b_v = av[:, :, 0, :], av[:, :, 1, ::-1]
                                    a_i, b_i = ai[:, :, 0, :], ai[:, :, 1, ::-1]
                                    o_lo_v, o_lo_i = ov[:, :, 1, ::-1], oi[:, :, 1, ::-1]
                                else:
                                    a_v, b_v = av[:, :, 0, :], av[:, :, 1, :]
                                    a_i, b_i = ai[:, :, 0, :], ai[:, :, 1, :]
                                    o_lo_v, o_lo_i = ov[:, :, 1, :], oi[:, :, 1, :]
                                nc.vector.tensor_tensor(cm, a_v, b_v, op=Alu.is_lt)
                                nc.vector.tensor_tensor(ov[:, :, 0, :], a_v, b_v, op=Alu.max)
                                nc.vector.tensor_tensor(o_lo_v, a_v, b_v, op=Alu.min)
                                nc.vector.tensor_copy(alt_i[:], cur_i[:])
                                nc.vector.copy_predicated(oi[:, :, 0, :], cm, b_i)
                                nc.vector.copy_predicated(o_lo_i, cm, a_i)
                                cur_v, alt_v = alt_v, cur_v
                                cur_i, alt_i = alt_i, cur_i
                            L *= 2
                        nc.sync.dma_start(
                            topk_o.rearrange("(st2 p) t -> st2 p t", p=128)[st], cur_i[:])
    nc.compile()
    return nc


_NC_CACHE = {}


def get_nc(cfg: Cfg):
    key = str(cfg)
    if key not in _NC_CACHE:
        _NC_CACHE[key] = build(cfg)
    return _NC_CACHE[key]


def make_in_maps(cfg: Cfg, inputs: dict):
    Ssh = cfg.Ssh
    hs = np.ascontiguousarray(np.asarray(inputs["hidden_states"])[0], dtype=np.float32)
    qr = np.ascontiguousarray(np.asarray(inputs["q_resid"])[0], dtype=np.float32)
    cosf = np.ascontiguousarray(np.asarray(inputs["cos"])[0, :, 0], dtype=np.float32)
    sinf = np.ascontiguousarray(np.asarray(inputs["sin"])[0, :, 0], dtype=np.float32)
    ones2 = np.zeros((128, 256), dtype=np.float32)
    ones2[0:64, 128] = 1.0
    ones2[64:128, 129] = 1.0
    com = dict(
        hidden=hs, cos_f=cosf, sin_f=sinf,
        Wq=np.ascontiguousarray(inputs["Wq"], dtype=np.float32),
        Wk=np.ascontiguousarray(inputs["Wk"], dtype=np.float32),
        Ww=np.ascontiguousarray(inputs["Ww"], dtype=np.float32),
        ln_g=np.ascontiguousarray(np.broadcast_to(
            np.asarray(inputs["ln_gamma"], dtype=np.float32), (128, cfg.D))),
        ln_b=np.ascontiguousarray(np.broadcast_to(
            np.asarray(inputs["ln_beta"], dtype=np.float32), (128, cfg.D))),
        ones2=ones2, ident=np.eye(128, dtype=np.float32),
    )
    maps = []
    for c in range(cfg.n_cores):
        sl = slice(c * Ssh, (c + 1) * Ssh)
        maps.append(dict(
            com,
            hidden_o=np.ascontiguousarray(hs[sl]),
            q_resid_o=np.ascontiguousarray(qr[sl]),
            cos_o=np.ascontiguousarray(cosf[sl]),
            sin_o=np.ascontiguousarray(sinf[sl]),
        ))
    return maps


def kernel(hidden_states, q_resid, cos, sin, Wq, Wk, ln_gamma, ln_beta, Ww,
           _want_trace=False):
    cfg = Cfg()
    inputs = dict(hidden_states=hidden_states, q_resid=q_resid, cos=cos, sin=sin,
                  Wq=Wq, Wk=Wk, ln_gamma=ln_gamma, ln_beta=ln_beta, Ww=Ww)
    in_maps = make_in_maps(cfg, inputs)
    nc = get_nc(cfg)
    res = run_bass_kernel_spmd(nc, in_maps, core_ids=list(range(cfg.n_cores)),
                               trace=_want_trace)
    out = np.concatenate([res.results[c]["topk_o"].astype(np.int32)
                          for c in range(cfg.n_cores)], axis=0)
    if _want_trace:
        kernel._last_trace = res
    return out[None]
